# revision 15
# baseline (speedup 1.0000x reference)
"""GAT/GCN message-passing layer on 8 Trainium2 NeuronCores.

Math: the reference computes, per query node i,
    e[i,j]   = f_src[i] + f_dst[j]           (GAT additive attention, masked by Ahat>0)
    attn     = softmax_j(e masked)
    out      = relu(attn @ h_prime)
The f_src[i] term is constant along the softmax axis, so it cancels:
    attn[i,j] = Ahat[i,j]*exp(f_dst[j]) / sum_k Ahat[i,k]*exp(f_dst[k])
With g = exp(f_dst) the whole layer is one GEMM over the adjacency:
    out = relu( (Ahat @ [g*h_prime | g])[:, :256] / (Ahat @ [g*h_prime | g])[:, 256:] )
where h_prime = node_feats @ w and f_dst = node_feats @ (w @ w_a @ a[2:4]).

Sharding: 1D row partition of query nodes.  Each of the 8 cores owns 1024 rows
of Ahat and computes its 1024x256 slice of the output.  The small tensors
(node_feats^T, [w | u]) are replicated; each core recomputes the
B = [g*h_prime | g] panel locally, which is cheaper than a collective.

The adjacency is binary, so fp8e4m3 is LOSSLESS for it: Ahat ships as fp8
(8.4 MB/core, half the bf16 bytes) pre-packed on the host into the exact SBUF
tile layout (fully-contiguous 8KB descriptor runs) and lives pinned in SBUF.
The PE matmuls mix fp8 stationary (adjacency) with bf16 moving (B panel); the
array runs at bf16 speed either way and fp8 weights load faster via FWL.

Schedule: everything DMA is SP-issued and completion-chained (params chunks
just ahead of the prefix's consumption, then the adjacency tiles just ahead
of the main stream), so the small early chunks never share fabric with bulk
loads.  A handful of throwaway matmuls at t=0 warm the PE HAM clock gate
(cold PE runs at 1.2 GHz for the first ~3.4us of activity).  The PE stream is
prefix -> main j<32 (banks 2..7) -> catch-up j<32 (banks 0/1) -> j>=32 (all),
then per-quad stops with interleaved epilogues so stores overlap the last
matmuls.  ACT computes exp(f_dst); DVE (otherwise idle) builds the B panel;
bf16 output goes out as two flat [128, 1024] stores (host reassembles).

Sync-wait discipline (walrus caps: 1 wait for most formats; bacc splits
excess waits into ~150ns event-semaphore preludes, so staying at <=1 is also
a performance matter): PSUM-bank readers are chained by the framework (even
same-engine), same-engine RAW is a sync wait, and waits are elided only when
the SAME engine already waited on that semaphore at >= the value.  Hence the
absorber reads, the reader orderings, and the SP-nop tail funnel below.
"""

import os
import sys

import ml_dtypes
import numpy as np

sys.path.insert(0, "/opt/trn_rl_repo")

import concourse.bass as bass  # noqa: E402
import concourse.tile as tile  # noqa: E402
from concourse import mybir  # noqa: E402
from concourse.bass_utils import run_bass_kernel_spmd  # noqa: E402
from concourse.tile import add_dep_helper  # noqa: E402

N = 8192
F = 256  # in_features == out_features
FE = F + 1  # h_prime columns + the g column
NCORES = 8
ROWS = N // NCORES  # 1024 output rows per core
P = 128
NJ = N // P  # 64 contraction blocks
NI = ROWS // P  # 8 output-row blocks per core

BF = mybir.dt.bfloat16
FP8 = mybir.dt.float8e4

PJ = 8  # j-blocks per pinned adjacency tile
NPIN = NJ // PJ  # 8 -- everything pinned
JSTART = 32  # banks 0/1 prefix-borrow boundary (caught up mid-stream)

# params flat layout: [wext (2*FE) | nfT (NJ*256)], one descriptor/partition
WCOLS = 2 * FE
PCOLS = WCOLS + NJ * 2 * P
# completion-chained chunk bounds (j-counts 2,10,16,18,18): each chunk lands
# just ahead of the prefix j that first needs it
PB = [0] + [WCOLS + nj * 2 * P for nj in (2, 12, 28, 46, NJ)]

_CACHE = {}


def _build():
    nc = bass.Bass(
        "TRN2",
        target_bir_lowering=False,
        debug=False,
        enable_asserts=True,
        num_devices=NCORES,
    )
    aT = nc.dram_tensor("aT", [P, NJ * ROWS], FP8, kind="ExternalInput").ap()
    params = nc.dram_tensor("params", [P, PCOLS], BF, kind="ExternalInput").ap()
    # output in SBUF-flat layout [p, i*F+f] = out[i*128+p, f]; host reassembles
    out = nc.dram_tensor("out", [P, NI * F], BF, kind="ExternalOutput").ap()

    with tile.TileContext(nc) as tc:
        _body(tc, aT, params, out)
    return nc


def _body(tc, aT, params, out):
    nc = tc.nc
    f32 = mybir.dt.float32
    Exp = mybir.ActivationFunctionType.Exp

    with (
        tc.tile_pool(name="consts", bufs=1) as consts,
        tc.tile_pool(name="opool", bufs=1) as opool,
        tc.tile_pool(name="rpool", bufs=8) as rpool,
        tc.tile_pool(name="psum", bufs=1, space="PSUM") as psum,
    ):
        # ---- loads ---------------------------------------------------------
        # SP-issued, completion-chained: c0..c4 then pin0..pin7.  Chaining
        # keeps each transfer at full fabric bandwidth and lands it just
        # ahead of its first consumer; SP program order provides the issue
        # ordering for free.
        params_sb = consts.tile([P, PCOLS], BF, tag="params")
        chain = None
        pchunks = []
        for c in range(len(PB) - 1):
            lo, hi = PB[c], PB[c + 1]
            d = nc.sync.dma_start(params_sb[:, lo:hi], params[:, lo:hi])
            if chain is not None:
                add_dep_helper(d.ins, chain.ins, reason="dma chain")
            chain = d
            pchunks.append(d)

        def wext_sb(kb):
            return params_sb[:, kb * FE : (kb + 1) * FE]

        def nfT_sb(j, kb):
            o = WCOLS + j * 2 * P + kb * P
            return params_sb[:, o : o + P]

        pinned = []
        pdmas = []
        for t in range(NPIN):
            pt = consts.tile([P, PJ * ROWS], FP8, tag=f"aTp{t}", name=f"aTp{t}")
            pinned.append(pt)
            d = nc.sync.dma_start(pt[:], aT[:, t * PJ * ROWS : (t + 1) * PJ * ROWS])
            add_dep_helper(d.ins, chain.ins, reason="dma chain")
            chain = d
            pdmas.append(d)

        def a_lhsT(j, i):
            """SBUF [128, 128] lhsT view of adjacency j-block, i-block i."""
            t = pinned[j // PJ]
            o = j % PJ
            return t[:, o * ROWS + i * P : o * ROWS + (i + 1) * P]

        # ---- PSUM accumulators --------------------------------------------
        acc = [
            psum.tile([P, FE], f32, tag=f"acc{i}", name=f"acc{i}") for i in range(NI)
        ]

        # ---- HAM warm-up ---------------------------------------------------
        # ~8 throwaway matmuls on zeroed SBUF keep the PE busy through the
        # cold first HAM window while the first params chunk is in flight, so
        # the prefix starts at 2.4 GHz instead of 1.2.
        warm_in = rpool.tile([P, FE], BF, tag="warm_in")
        wm = nc.gpsimd.memset(warm_in[:], 0.0)
        for k in range(14):
            wmm = nc.tensor.matmul(
                acc[NI - 1][:],
                lhsT=warm_in[:, 0:P],
                rhs=warm_in[:],
                start=True,
                stop=True,
            )
            if k == 0:
                add_dep_helper(wmm.ins, wm.ins, reason="warmup after memset")

        # ---- prefix: B[j] = [g*h_prime | g], all 64 j-blocks ---------------
        # h' matmuls borrow PSUM banks 0/1; those banks' main accumulation is
        # caught up mid-stream so every bank stops at j=63.  hp is read only
        # by DVE (PSUM reader-chain): cp takes the PE wait, ACT exps from
        # SBUF (DVE wait), the absorber ab takes the ACT wait, and the
        # broadcast mul's remaining sync wait is the reader-chain tick on cp.
        B_all = consts.tile([P, NJ * FE], BF, tag="B")
        btile = [B_all[:, j * FE : (j + 1) * FE] for j in range(NJ)]
        G = consts.tile([P, NJ], f32, tag="G")
        scr = consts.tile([P, 8], f32, tag="scr")
        scr2 = consts.tile([P, 8], f32, tag="scr2")
        prev_act = None
        prev_dve = None
        for j in range(NJ):
            hp = acc[j % 2]
            for kb in range(2):
                nc.tensor.matmul(
                    hp[:],
                    lhsT=nfT_sb(j, kb),
                    rhs=wext_sb(kb),
                    start=(kb == 0),
                    stop=(kb == 1),
                )
            b = btile[j]
            gj = G[:, j : j + 1]
            fc = scr[:, j % 8 : j % 8 + 1]
            cp = nc.vector.tensor_copy(fc, hp[:, F : F + 1])
            if prev_dve is not None:
                add_dep_helper(cp.ins, prev_dve.ins, sync=False, reason="dve order")
            ex = nc.scalar.activation(gj, fc, Exp)
            if prev_act is not None:
                add_dep_helper(ex.ins, prev_act.ins, sync=False, reason="act order")
            prev_act = ex
            ab = nc.vector.tensor_copy(scr2[:, j % 8 : j % 8 + 1], gj)
            add_dep_helper(ab.ins, cp.ins, sync=False, reason="dve order")
            mu = nc.vector.tensor_mul(
                b[:, 0:F], hp[:, 0:F], gj.broadcast_to([P, F])
            )
            add_dep_helper(mu.ins, ab.ins, sync=False, reason="dve order")
            prev_dve = mu
            if j % 8 == 7:
                # strided copy drops this 8-group's g column into B (DVE, so
                # the main matmuls' B dependency stays single-engine)
                c0 = j - 7
                prev_dve = nc.vector.tensor_copy(
                    B_all[:, c0 * FE + F : (j + 1) * FE : FE], G[:, c0 : j + 1]
                )

        # ---- main stream ---------------------------------------------------
        # phase A: j<32 for banks 2..7; phase B: catch-up j<32 for banks 0/1;
        # phase C: j in [32, 62] for all banks; phase D: per-bank j=63 stops
        # with quad epilogues interleaved (stores overlap the last matmuls).
        def mm(i, j, start, stop):
            return nc.tensor.matmul(
                acc[i][:],
                lhsT=a_lhsT(j, i),
                rhs=btile[j][:],
                start=start,
                stop=stop,
            )

        last_mm = None
        for j in range(JSTART):
            for i in range(2, NI):
                last_mm = mm(i, j, j == 0, False)
        for j in range(JSTART):
            for i in range(2):
                last_mm = mm(i, j, j == 0, False)
        for j in range(JSTART, NJ - 1):
            for i in range(NI):
                last_mm = mm(i, j, False, False)

        # ---- phase D + epilogue -------------------------------------------
        # out[i] = relu(acc[i][:, :F] / acc[i][:, F]).  Stops in bank order
        # 0..7; epilogue quads process banks [2,0,1,3] and [4,5,6,7]: bank 2
        # first so its fresh PE wait covers banks 0/1 (whose denom copies
        # carry the DVE reader-chain wait from the prefix instead).
        otile = opool.tile([P, NI * F], BF, tag="o")
        denom = rpool.tile([P, NI], f32, tag="denom")
        recip = rpool.tile([P, NI], f32, tag="recip")
        sac = rpool.tile([P, NI], f32, tag="sac")
        stores = []
        act_last = prev_act
        dve_last = prev_dve

        for i in range(4):
            last_mm = mm(i, NJ - 1, False, True)
        quads = [[2, 0, 1, 3], [4, 5, 6, 7]]
        for q, banks in enumerate(quads):
            if q == 1:
                for i in banks:
                    last_mm = mm(i, NJ - 1, False, True)
            for i in banks:
                dc = nc.scalar.copy(denom[:, i : i + 1], acc[i][:, F : F + 1])
                add_dep_helper(dc.ins, act_last.ins, sync=False, reason="act order")
                act_last = dc
            qs = slice(4 * q, 4 * q + 4)
            rec = nc.vector.reciprocal(recip[:, qs], denom[:, qs])
            add_dep_helper(rec.ins, dve_last.ins, sync=False, reason="dve order")
            dve_last = rec
            sa = nc.scalar.copy(sac[:, qs], recip[:, qs])
            add_dep_helper(sa.ins, act_last.ins, sync=False, reason="act order")
            act_last = sa
            for i in banks:
                o = otile[:, i * F : (i + 1) * F]
                rl = nc.scalar.activation(
                    o,
                    acc[i][:, 0:F],
                    mybir.ActivationFunctionType.Relu,
                    scale=recip[:, i : i + 1],
                )
                add_dep_helper(rl.ins, act_last.ins, sync=False, reason="act order")
                act_last = rl
            st = nc.gpsimd.dma_start(
                out[:, 4 * q * F : (4 * q + 4) * F], otile[:, 4 * q * F : (4 * q + 4) * F]
            )
            add_dep_helper(st.ins, act_last.ins, reason="store after relu")
            stores.append(st)

        # Funnel every proc's final tick into SP via single-wait nops so the
        # kernel-tail drain (>=1-wait cap) has nothing left to wait on.  DMAs
        # fan out over several HW-DGE queues, so DMA deps get two nops each.
        deps = []
        for d in [*pdmas, *pchunks]:
            deps += [d, d]
        deps += [*stores, *stores, last_mm, act_last, dve_last]
        for dep in deps:
            nop = nc.sync.nop(nofuse=True, hint="tail_funnel")
            add_dep_helper(nop.ins, dep.ins, reason="tail funnel")


def _prep_inputs(node_feats, Ahat, w, w_a, a):
    node_feats = np.asarray(node_feats, dtype=np.float32)
    Ahat = np.asarray(Ahat, dtype=np.float32)
    w = np.asarray(w, dtype=np.float32)
    w_a = np.asarray(w_a, dtype=np.float32)
    a = np.asarray(a, dtype=np.float32)

    u = w @ (w_a @ a[2:4])  # [256, 1]
    wext = np.concatenate([w, u], axis=1)  # [256, 257]
    # flat params, partition-major: [wext (kb,FE) | nfT (j, kb, c)]
    wext_f = wext.reshape(2, P, FE).transpose(1, 0, 2).reshape(P, 2 * FE)
    nfT = node_feats.T  # [256, 8192]
    nfT_f = (
        nfT.reshape(2, P, NJ, P).transpose(1, 2, 0, 3).reshape(P, NJ * 2 * P)
    )
    params = np.ascontiguousarray(
        np.concatenate([wext_f, nfT_f], axis=1)
    ).astype("bfloat16")

    in_maps = []
    for c in range(NCORES):
        # fp8 adjacency slice, packed into the exact SBUF layout:
        # aT_flat[p, jb*ROWS + c] = A[row0+c, jb*128 + p]
        aT_c = Ahat[c * ROWS : (c + 1) * ROWS, :].T.astype(ml_dtypes.float8_e4m3)
        aT_c = np.ascontiguousarray(
            aT_c.reshape(NJ, P, ROWS).transpose(1, 0, 2).reshape(P, NJ * ROWS)
        )
        in_maps.append({"aT": aT_c, "params": params})
    return in_maps


def _run(inputs, trace=False, **kwargs):
    if "nc" not in _CACHE:
        _CACHE["nc"] = _build()
    nc = _CACHE["nc"]
    in_maps = _prep_inputs(**inputs)
    res = run_bass_kernel_spmd(
        nc, in_maps, core_ids=list(range(NCORES)), trace=trace, **kwargs
    )
    # out is [P, NI*F] bf16 per core in SBUF-flat layout; reassemble + upcast
    full = np.concatenate(
        [
            res.results[c]["out"]
            .astype(np.float32)
            .reshape(P, NI, F)
            .transpose(1, 0, 2)
            .reshape(ROWS, F)
            for c in range(NCORES)
        ],
        axis=0,
    )
    return full, res


def kernel(**inputs) -> np.ndarray:
    out, _ = _run(inputs, trace=False)
    return out
